# revision 58
# baseline (speedup 1.0000x reference)
"""Trainium2 Bass kernel for RFA causal linear attention (nn_CausalAttention).

Reference computation (T=1024, B=4, E=512, H=8, D=64, P=64):
  q,k,v = x @ W{q,k,v}.T + b          (biases are zero in setup_inputs)
  phi(u)_h = [sin, cos](rm_h @ (u_h / max(||u_h||, eps))) / sqrt(P)
  causal linear attention scan over t:  S += phi_k v^T ; z += phi_k
  attn_t = (phi_q . S) / max(phi_q . z, eps);  out = attn @ Wo.T + b

Sharding (collective-free): 8 cores = 4 batches x 2 head-groups of 4
heads. Each core runs the FULL T=1024 chunked scan (C=128, 8 chunks)
for its heads, so the sequential state never crosses cores and there is
no collective. The output projection contracts over all 8 heads, so each
core emits the partial out = attn_g @ Wo.T[rows_g]; the host adds the
two group partials per batch while unsharding.

Engine layout: PE does matmuls and the feature/attn transposes (pair-
batched PSUM tiles, one evacuation copy per pair). The norm scale is
one Abs_reciprocal_sqrt ACT op per chunk (same activation-table set as
Square/Copy). DVE does the fused scale-multiply+range-wrap (custom op),
reductions, mask, and the qs/qz normalization; copies are split across
DVE/ACT. Outputs are bf16 partials (summed in f32 on the host), halving
the output DMA. Input DMAs are spread across the sync/scalar/pool
queues (per-queue bandwidth is the constraint) with the first xt tile
split fine so the first matmul starts as soon as the NEFF prologue
ends.

phi features carry no P**-0.5 factor; it cancels in qs/qz except at the
eps clamp, handled by scaling qz by 1/P before the clamp (SCALE_QZ).
All matmuls run in bf16 with f32 PSUM accumulation.
"""
import math

import numpy as np
import ml_dtypes

BF16 = ml_dtypes.bfloat16

# problem dims
T, B, E, H, D, P = 1024, 4, 512, 8, 64, 64
HG = 4              # heads per core
K = 2 * P           # feature dim per head = 128
C = 128             # chunk size
NCH = T // C        # chunks per core = 8
EC = 4              # contract chunks of 128 over E
OC = 2              # contract chunks of 128 over my heads' hd for Wo
EPS = 1e-6
DAUG = D + 1        # value dim augmented with ones column (for z)
VSTR = 68           # padded stride for Vaug tiles
N_CORES = 8
SCALE_QZ = 1.0 / P  # two factors of P**-0.5 folded into the qz clamp

_CACHE = {}


def _mult_range_wrap_op():
    """Custom DVE op: out = wrap(in0*in1 + s0) into [-s1, s1] by one period
    2*s1 — fuses the feature scale-multiply with the sin range reduction."""
    import numpy as np
    import concourse.dve_ops as dve_ops
    from concourse.dve_spec import C0, C1, C2, Src0, Src1, lower
    from concourse.dve_uop import DveOpSpec

    name = "MULT_RANGE_WRAP_ANT"
    for op in dve_ops.OPS:
        if op.name == name:
            return op

    def _ref(in0, in1, s0, s1, imm2):
        y = (in0.astype(np.float32)
             * np.asarray(in1, np.float32).reshape(in0.shape) + s0)
        return y + (s1 + s1) * (
            (y < -s1).astype(np.float32) - (y > s1).astype(np.float32))

    # period = 2*bound = C1 + C1 (no imm2 slot with a 2D src1)
    y = Src0 * Src1 + C0
    op = dve_ops.DveOp(
        name,
        dve_ops.Spec(body=y + (C1 + C1) * ((y < -C1) - (y > C1)),
                     reference=_ref),
        subdim=False,
        uops_sha={},
    )
    dve_ops.OPS.append(op)
    dve_ops.CUSTOM_DVE_SPECS[name] = op.spec
    dve_ops._SUB_OPCODE_FOR_NAME[name] = (dve_ops._CUSTOM_DVE_ROW_BASE
                                          + len(dve_ops.OPS) - 1)
    from concourse.dve_ops import get_dve_sub_opcode
    for ver in ("v3", "v4"):
        spec_l = DveOpSpec(name=name, opcode=get_dve_sub_opcode(name),
                           uops=lower(op.spec, ver=ver),
                           rd1_en=dve_ops.has_src1(op.spec))
        op.uops_sha[ver] = spec_l.sha(ver)
    return op


def _build(n_cores, dbg=False):
    import concourse.bass as bass
    import concourse.tile as tile
    from concourse import bacc, mybir
    from concourse.bass import ts

    dt = mybir.dt
    AF = mybir.ActivationFunctionType
    ALU = mybir.AluOpType
    MRW = _mult_range_wrap_op()

    nc = bacc.Bacc("TRN2", target_bir_lowering=False, debug=False,
                   num_devices=n_cores)

    def din(name, shape, dtype=dt.bfloat16):
        return nc.dram_tensor(name, shape, dtype, kind="ExternalInput").ap()

    xt_d = din("xt", [128, EC, T])               # x^T tiled: [p, ec, t]
    wk2_d = din("wk2", [128, EC, 512])           # rhs tiles of [Wk.T|Wke.T]
    wq3_d = din("wq3", [128, EC, 768])           # rhs tiles of [Wq.T|Wqe.T|Wv.T]
    wo_d = din("wo", [128, OC, E])               # rhs tiles of Wo.T (my hd rows)
    mk_d = din("mask", [128, 128])
    id_d = din("ident", [128, 128])
    out_d = nc.dram_tensor("out", [128, NCH, E], dt.bfloat16,
                           kind="ExternalOutput").ap()

    with tile.TileContext(nc) as tc:
        with (tc.tile_pool(name="consts", bufs=1) as cpool,
              tc.tile_pool(name="scratch", bufs=6) as spool,
              tc.tile_pool(name="pA", bufs=6, space="PSUM") as pA,
              tc.tile_pool(name="pB", bufs=2, space="PSUM") as pB):
            # ---- persistent SBUF tensors ----
            xt = cpool.tile([128, EC, T], dt.bfloat16, tag="xt")
            wk2 = cpool.tile([128, EC, 512], dt.bfloat16, tag="wk2")
            wq3 = cpool.tile([128, EC, 768], dt.bfloat16, tag="wq3")
            wo = cpool.tile([128, OC, E], dt.bfloat16, tag="wo")
            mask = cpool.tile([128, 128], dt.bfloat16, tag="mask")
            ident = cpool.tile([128, 128], dt.bfloat16, tag="ident")
            n2 = cpool.tile([128, NCH, 2, HG], dt.float32, tag="n2")
            scal = cpool.tile([128, NCH, 2, HG], dt.float32, tag="scal")
            vsb = cpool.tile([128, NCH, HG, VSTR], dt.bfloat16, tag="vsb")
            pk_nat = cpool.tile([128, NCH, HG, K], dt.bfloat16, tag="pk_nat")
            pq_nat = cpool.tile([128, NCH, HG, K], dt.bfloat16, tag="pq_nat")
            pkt = cpool.tile([128, NCH, HG, C], dt.bfloat16, tag="pkt")
            pqt = cpool.tile([128, NCH, HG, C], dt.bfloat16, tag="pqt")
            s_snap = cpool.tile([128, NCH, HG, VSTR], dt.bfloat16, tag="s_snap")
            attnT = cpool.tile([128, NCH, OC, C], dt.bfloat16, tag="attnT")
            out_sb = cpool.tile([128, 2, E], dt.bfloat16, tag="out_sb")

            # ---- input DMAs, spread across engine queues so the issue
            # serialization (~0.6us each) parallelizes; K-side first ----
            # first-tile loads interleaved across the three issue queues so
            # the ~0.6us-per-issue serialization doesn't gate the first
            # matmul: sync takes xt c0/c1 quarters, gpsimd xt c2/c3, scalar
            # wk2 — all in parallel right after the NEFF prologue.
            nc.sync.dma_start(xt[:, 0, 0:128], xt_d[:, 0, 0:128])
            nc.sync.dma_start(xt[:, 1, 0:128], xt_d[:, 1, 0:128])
            nc.gpsimd.dma_start(xt[:, 2, 0:128], xt_d[:, 2, 0:128])
            nc.gpsimd.dma_start(xt[:, 3, 0:128], xt_d[:, 3, 0:128])
            for c in range(EC):
                nc.scalar.dma_start(wk2[:, c, :], wk2_d[:, c, :])
            for c in range(EC):
                nc.gpsimd.dma_start(wq3[:, c, :], wq3_d[:, c, :])
            for c in range(EC):
                nc.sync.dma_start(xt[:, c, 128:512], xt_d[:, c, 128:512])
            for c in range(EC):
                nc.sync.dma_start(xt[:, c, 512:1024], xt_d[:, c, 512:1024])
            nc.scalar.dma_start(wo[:], wo_d[:])
            # only the augmented ones-column needs presetting: cols 0:D are
            # overwritten by the v copies and cols D+1.. are never read
            nc.gpsimd.memset(vsb[:, :, :, D:DAUG], 1.0)
            nc.gpsimd.dma_start(ident[:], id_d[:])
            nc.gpsimd.dma_start(mask[:], mk_d[:])

            def emit_proj(m, psV):
                psK = pA.tile([128, 512], dt.float32, tag="pb1")
                for c in range(EC):
                    nc.tensor.matmul(psK[:], xt[:, c, ts(m, 128)],
                                     wk2[:, c, :],
                                     start=(c == 0), stop=(c == EC - 1))
                psQ = pB.tile([128, 512], dt.float32, tag="pb2")
                for c in range(EC):
                    nc.tensor.matmul(psQ[:], xt[:, c, ts(m, 128)],
                                     wq3[:, c, 0:512],
                                     start=(c == 0), stop=(c == EC - 1))
                for c in range(EC):
                    nc.tensor.matmul(psV[:, m % 2, :], xt[:, c, ts(m, 128)],
                                     wq3[:, c, 512:768],
                                     start=(c == 0), stop=(c == EC - 1))
                # squared norms for both sides -> one reduce -> one-op scale.
                # Abs_reciprocal_sqrt shares its ACT table set with Square,
                # so the norm path adds no DVE work, and the per-chunk chain
                # frees psK/psQ without waiting on the pair partner.
                sq = spool.tile([128, 2, HG * D], dt.float32, tag="sq")
                nc.scalar.activation(sq[:, 0, :], psK[:, 0:256], AF.Square)
                nc.scalar.activation(sq[:, 1, :], psQ[:, 0:256], AF.Square)
                nc.vector.tensor_reduce(
                    n2[:, m, :, :],
                    sq[:].rearrange("p s (h d) -> p s h d", h=HG),
                    mybir.AxisListType.X, ALU.add)
                return psK, psQ

            # rsqrt on DVE only: rational seed 1/(a*n2+b) (within 8.4% of
            # n2**-0.5 over n2 in [25, 130], convergent for any n2 > 0) plus
            # two Newton steps, pair-batched 16-element ops. Keeping rsqrt
            # off ACT leaves ACT with only Square/Sin/Copy, which share one
            # activation-table set: the 2x-per-pair 1.3us table reloads of
            # the Sqrt/Abs_reciprocal_sqrt variants disappear.
            def scales_pair(m0):
                v = n2[:, m0:m0 + 2, :, :]
                r = scal[:, m0:m0 + 2, :, :]
                nc.vector.tensor_scalar(r, v, 0.061, 3.476, ALU.mult, ALU.add)
                nc.vector.reciprocal(r, r)
                for it in range(2):
                    nt = spool.tile([128, 2, 2, HG], dt.float32, tag="nt")
                    nc.vector.tensor_tensor(nt[:], r, r, ALU.mult)
                    nc.vector.tensor_tensor(nt[:], nt[:], v, ALU.mult)
                    nc.vector.tensor_scalar(nt[:], nt[:], -0.5, 1.5,
                                            ALU.mult, ALU.add)
                    nc.vector.tensor_tensor(r, r, nt[:], ALU.mult)

            # scaled projection -> wrapped phases for chunk m (both shifts).
            # wrap(proj*scale + shift) into [-pi, pi] in one fused DVE op.
            def wraps(m, side, ps, u):
                src = ps.rearrange("p (h q) -> p h q", h=HG)
                sc_b = (scal[:, m, side, :].unsqueeze(2)
                        .to_broadcast((128, HG, P)))
                nc.vector._custom_dve(MRW, out=u[:, 0, m % 2, :, :], in0=src,
                                      in1=sc_b, s0=0.0, s1=math.pi)
                nc.vector._custom_dve(MRW, out=u[:, 1, m % 2, :, :], in0=src,
                                      in1=sc_b, s0=math.pi / 2, s1=math.pi)

            # one batched Sin per phase-half for a chunk pair
            def sins(m0, u, feat):
                for s in range(2):
                    dst = feat[:, m0:m0 + 2, :, ts(s, P)]
                    nc.scalar.activation(dst, u[:, s, :, :, :], AF.Sin)

            def emit_feats_pair(m0, uk, uq, psV):
                sins(m0, uk, pk_nat)
                sins(m0, uq, pq_nat)
                nc.scalar.copy(
                    vsb[:, m0:m0 + 2, :, 0:D],
                    psV[:].rearrange("p c (h d) -> p c h d", h=HG))

            # PE transposes (pair-batched copies) + state/A matmuls
            def emit_tr_pair(m0):
                trK = pA.tile([128, 1024], dt.bfloat16, tag="pb1")
                for c in range(2):
                    for h in range(HG):
                        nc.tensor.transpose(trK[:, ts(c * HG + h, 128)],
                                            pk_nat[:, m0 + c, h, :], ident[:])
                nc.vector.tensor_copy(
                    pkt[:, m0:m0 + 2, :, :],
                    trK[:].rearrange("p (c h t) -> p c h t", c=2, h=HG))
                trQ = pA.tile([128, 1024], dt.bfloat16, tag="pb1")
                for c in range(2):
                    for h in range(HG):
                        nc.tensor.transpose(trQ[:, ts(c * HG + h, 128)],
                                            pq_nat[:, m0 + c, h, :], ident[:])
                nc.scalar.copy(
                    pqt[:, m0:m0 + 2, :, :],
                    trQ[:].rearrange("p (c h t) -> p c h t", c=2, h=HG))

            def emit_mid_pair(m0):
                atm2 = []
                for m in (m0, m0 + 1):
                    psS = pA.tile([128, HG, 128], dt.float32, tag="pb1")
                    for h in range(HG):
                        nc.tensor.matmul(psS[:, h, 0:DAUG], pk_nat[:, m, h, :],
                                         vsb[:, m, h, 0:DAUG],
                                         start=(h == 0), stop=(h == HG - 1))
                    if m == 0:
                        nc.vector.tensor_copy(s_snap[:, 0, :, 0:DAUG],
                                              psS[:, :, 0:DAUG])
                    else:
                        nc.vector.scalar_tensor_tensor(
                            s_snap[:, m, :, 0:DAUG], psS[:, :, 0:DAUG], 1.0,
                            s_snap[:, m - 1, :, 0:DAUG], ALU.mult, ALU.add)
                    psA = pA.tile([128, HG, 128], dt.float32, tag="pb1")
                    for h in range(HG):
                        nc.tensor.matmul(psA[:, h, :], pkt[:, m, h, :],
                                         pqt[:, m, h, :],
                                         start=(h == 0), stop=(h == HG - 1))
                    atm = spool.tile([128, HG, C], dt.bfloat16, tag="atm")
                    mk_b = mask[:].unsqueeze(1).to_broadcast((128, HG, C))
                    nc.vector.tensor_tensor(atm[:], psA[:], mk_b, ALU.mult)
                    atm2.append(atm)
                return atm2

            # qs + attn + output projection for a chunk pair
            def emit_out_pair(m0, atm2):
                attns = []
                for m, atm in zip((m0, m0 + 1), atm2):
                    psq = pA.tile([128, HG, 128], dt.float32, tag="pb1")
                    for h in range(HG):
                        nc.tensor.matmul(psq[:, h, 0:DAUG], atm[:, h, :],
                                         vsb[:, m, h, 0:DAUG],
                                         start=(h == 0),
                                         stop=(m == 0 and h == HG - 1))
                    if m > 0:
                        for h in range(HG):
                            nc.tensor.matmul(psq[:, h, 0:DAUG],
                                             pqt[:, m, h, :],
                                             s_snap[:, m - 1, h, 0:DAUG],
                                             start=False, stop=(h == HG - 1))
                    qz = spool.tile([128, HG], dt.float32, tag="qz")
                    nc.vector.tensor_scalar(qz[:], psq[:, :, D], SCALE_QZ,
                                            EPS, ALU.mult, ALU.max)
                    nc.vector.reciprocal(qz[:], qz[:])
                    attn = spool.tile([128, HG * D], dt.bfloat16, tag="attn")
                    qz_b = qz[:].unsqueeze(2).to_broadcast((128, HG, D))
                    nc.vector.scalar_tensor_tensor(
                        attn[:].rearrange("p (h d) -> p h d", h=HG),
                        psq[:, :, 0:D], SCALE_QZ, qz_b, ALU.mult, ALU.mult)
                    attns.append(attn)
                trA = pA.tile([128, 1024], dt.bfloat16, tag="pb1")
                for c in range(2):
                    for j in range(OC):
                        nc.tensor.transpose(trA[:, ts(c * OC + j, 128)],
                                            attns[c][:, ts(j, 128)], ident[:])
                nc.vector.tensor_copy(
                    attnT[:, m0:m0 + 2, :, :],
                    trA[:, 0:512].rearrange("p (c j t) -> p c j t", c=2, j=OC))
                for m in (m0, m0 + 1):
                    psO = pA.tile([128, E], dt.float32, tag="pb1")
                    for j in range(OC):
                        nc.tensor.matmul(psO[:], attnT[:, m, j, :],
                                         wo[:, j, :],
                                         start=(j == 0), stop=(j == OC - 1))
                    if m % 2 == 0:
                        nc.vector.tensor_copy(out_sb[:, 0, :], psO[:])
                    else:
                        nc.scalar.copy(out_sb[:, 1, :], psO[:])
                    nc.sync.dma_start(out_d[:, m, :], out_sb[:, m % 2, :])

            # ---- software-pipelined emission, chunk-pair granular.
            # Each engine queue sees the PREVIOUS pairs' mid/out work before
            # this pair's scales/feats, so the cross-engine norm-chain ladder
            # doesn't head-of-line-block already-ready work. ----
            atms = {}
            for i in range(NCH // 2 + 2):
                projs = None
                if i < NCH // 2:
                    m0 = 2 * i
                    psV = pA.tile([128, 2, 256], dt.float32, tag="pb1")
                    uk = spool.tile([128, 2, 2, HG, P], dt.float32, tag="uk")
                    uq = spool.tile([128, 2, 2, HG, P], dt.float32, tag="uq")
                    pa = emit_proj(m0, psV)
                    pb = emit_proj(m0 + 1, psV)
                    if i >= 1:
                        emit_tr_pair(2 * (i - 1))
                    scales_pair(m0)
                    wraps(m0, 0, pa[0][:, 256:512], uk)
                    wraps(m0, 1, pa[1][:, 256:512], uq)
                    wraps(m0 + 1, 0, pb[0][:, 256:512], uk)
                    wraps(m0 + 1, 1, pb[1][:, 256:512], uq)
                    projs = (m0, uk, uq, psV)
                elif i == NCH // 2:
                    emit_tr_pair(2 * (i - 1))
                if 1 <= i <= NCH // 2:
                    atms[2 * (i - 1)] = emit_mid_pair(2 * (i - 1))
                if i >= 2:
                    m0 = 2 * (i - 2)
                    emit_out_pair(m0, atms.pop(m0))
                if projs is not None:
                    emit_feats_pair(*projs)

    nc.compile()
    return nc


def _host_prep(x, random_matrices, Wq, Wk, Wv, Wo, n_cores):
    """Build per-core input maps: core = (batch, head-group of 4)."""
    rm = random_matrices

    def tile_w(M):  # [contract 512, out] -> [128, 4, out] bf16
        return np.ascontiguousarray(
            M.reshape(EC, 128, M.shape[1]).transpose(1, 0, 2)).astype(BF16)

    mask = np.triu(np.ones((128, 128), np.float32)).astype(BF16)
    ident = np.eye(128, dtype=BF16)
    in_maps = []
    for core in range(n_cores):
        b, g = core // 2, core % 2
        rows = slice(g * HG * D, (g + 1) * HG * D)
        Wqe = np.concatenate(
            [rm[g * HG + h] @ Wq[(g * HG + h) * D:(g * HG + h + 1) * D, :]
             for h in range(HG)], axis=0)          # [256, 512]
        Wke = np.concatenate(
            [rm[g * HG + h] @ Wk[(g * HG + h) * D:(g * HG + h + 1) * D, :]
             for h in range(HG)], axis=0)
        wk2 = tile_w(np.concatenate([Wk[rows, :].T, Wke.T], axis=1))
        wq3 = tile_w(np.concatenate(
            [Wq[rows, :].T, Wqe.T, Wv[rows, :].T], axis=1))
        woT = Wo[:, rows].T                        # [256, 512]
        wo = np.ascontiguousarray(
            woT.reshape(OC, 128, E).transpose(1, 0, 2)).astype(BF16)
        xl = np.ascontiguousarray(x[:, b, :].T)    # [512, 1024]
        in_maps.append({"xt": tile_w(xl), "wk2": wk2, "wq3": wq3, "wo": wo,
                        "mask": mask, "ident": ident})
    return in_maps


def kernel(x, random_matrices, Wq, bq, Wk, bk, Wv, bv, Wo, bo):
    x = np.asarray(x, np.float32)
    random_matrices = np.asarray(random_matrices, np.float32)
    Wq = np.asarray(Wq, np.float32)
    Wk = np.asarray(Wk, np.float32)
    Wv = np.asarray(Wv, np.float32)
    Wo = np.asarray(Wo, np.float32)
    assert (np.all(np.asarray(bq) == 0) and np.all(np.asarray(bk) == 0)
            and np.all(np.asarray(bv) == 0) and np.all(np.asarray(bo) == 0)), \
        "kernel specialized for zero biases (as in setup_inputs)"

    from concourse.bass_utils import run_bass_kernel_spmd

    if "nc" not in _CACHE:
        _CACHE["nc"] = _build(N_CORES)
    nc = _CACHE["nc"]

    in_maps = _host_prep(x, random_matrices, Wq, Wk, Wv, Wo, N_CORES)
    res = run_bass_kernel_spmd(nc, in_maps, core_ids=list(range(N_CORES)))

    out = np.empty((T, B, E), np.float32)
    for b in range(B):
        o0 = np.asarray(res.results[2 * b]["out"], np.float32)
        o1 = np.asarray(res.results[2 * b + 1]["out"], np.float32)
        out[:, b, :] = (o0 + o1).transpose(1, 0, 2).reshape(T, E)
    return out


# revision 61
# speedup vs baseline: 1.0355x; 1.0355x over previous
"""Trainium2 Bass kernel for RFA causal linear attention (nn_CausalAttention).

Reference computation (T=1024, B=4, E=512, H=8, D=64, P=64):
  q,k,v = x @ W{q,k,v}.T + b          (biases are zero in setup_inputs)
  phi(u)_h = [sin, cos](rm_h @ (u_h / max(||u_h||, eps))) / sqrt(P)
  causal linear attention scan over t:  S += phi_k v^T ; z += phi_k
  attn_t = (phi_q . S) / max(phi_q . z, eps);  out = attn @ Wo.T + b

Sharding (collective-free): 8 cores = 4 batches x 2 head-groups of 4
heads. Each core runs the FULL T=1024 chunked scan (C=128, 8 chunks)
for its heads, so the sequential state never crosses cores and there is
no collective. The output projection contracts over all 8 heads, so each
core emits the partial out = attn_g @ Wo.T[rows_g]; the host adds the
two group partials per batch while unsharding.

Engine layout: PE does matmuls and the feature/attn transposes (pair-
batched PSUM tiles, one evacuation copy per pair). The norm scale is
one Abs_reciprocal_sqrt ACT op per chunk (same activation-table set as
Square/Copy). DVE does the fused scale-multiply+range-wrap (custom op),
reductions, mask, and the qs/qz normalization; copies are split across
DVE/ACT. Outputs are bf16 partials (summed in f32 on the host), halving
the output DMA. Input DMAs are spread across the sync/scalar/pool
queues (per-queue bandwidth is the constraint) with the first xt tile
split fine so the first matmul starts as soon as the NEFF prologue
ends.

phi features carry no P**-0.5 factor; it cancels in qs/qz except at the
eps clamp, handled by scaling qz by 1/P before the clamp (SCALE_QZ).
All matmuls run in bf16 with f32 PSUM accumulation.
"""
import math

import numpy as np
import ml_dtypes

BF16 = ml_dtypes.bfloat16

# problem dims
T, B, E, H, D, P = 1024, 4, 512, 8, 64, 64
HG = 4              # heads per core
K = 2 * P           # feature dim per head = 128
C = 128             # chunk size
NCH = T // C        # chunks per core = 8
EC = 4              # contract chunks of 128 over E
OC = 2              # contract chunks of 128 over my heads' hd for Wo
EPS = 1e-6
DAUG = D + 1        # value dim augmented with ones column (for z)
VSTR = 68           # padded stride for Vaug tiles
N_CORES = 8
SCALE_QZ = 1.0 / P  # two factors of P**-0.5 folded into the qz clamp

_CACHE = {}


def _mult_range_wrap_op():
    """Custom DVE op: out = wrap(in0*in1 + s0) into [-s1, s1] by one period
    2*s1 — fuses the feature scale-multiply with the sin range reduction."""
    import numpy as np
    import concourse.dve_ops as dve_ops
    from concourse.dve_spec import C0, C1, C2, Src0, Src1, lower
    from concourse.dve_uop import DveOpSpec

    name = "MULT_RANGE_WRAP_ANT"
    for op in dve_ops.OPS:
        if op.name == name:
            return op

    def _ref(in0, in1, s0, s1, imm2):
        y = (in0.astype(np.float32)
             * np.asarray(in1, np.float32).reshape(in0.shape) + s0)
        return y + (s1 + s1) * (
            (y < -s1).astype(np.float32) - (y > s1).astype(np.float32))

    # period = 2*bound = C1 + C1 (no imm2 slot with a 2D src1)
    y = Src0 * Src1 + C0
    op = dve_ops.DveOp(
        name,
        dve_ops.Spec(body=y + (C1 + C1) * ((y < -C1) - (y > C1)),
                     reference=_ref),
        subdim=False,
        uops_sha={},
    )
    dve_ops.OPS.append(op)
    dve_ops.CUSTOM_DVE_SPECS[name] = op.spec
    dve_ops._SUB_OPCODE_FOR_NAME[name] = (dve_ops._CUSTOM_DVE_ROW_BASE
                                          + len(dve_ops.OPS) - 1)
    from concourse.dve_ops import get_dve_sub_opcode
    for ver in ("v3", "v4"):
        spec_l = DveOpSpec(name=name, opcode=get_dve_sub_opcode(name),
                           uops=lower(op.spec, ver=ver),
                           rd1_en=dve_ops.has_src1(op.spec))
        op.uops_sha[ver] = spec_l.sha(ver)
    return op


def _build(n_cores, dbg=False):
    import concourse.bass as bass
    import concourse.tile as tile
    from concourse import bacc, mybir
    from concourse.bass import ts

    dt = mybir.dt
    AF = mybir.ActivationFunctionType
    ALU = mybir.AluOpType
    MRW = _mult_range_wrap_op()

    nc = bacc.Bacc("TRN2", target_bir_lowering=False, debug=False,
                   num_devices=n_cores)

    def din(name, shape, dtype=dt.bfloat16):
        return nc.dram_tensor(name, shape, dtype, kind="ExternalInput").ap()

    xt_d = din("xt", [128, EC, T])               # x^T tiled: [p, ec, t]
    wk2_d = din("wk2", [128, EC, 512])           # rhs tiles of [Wk.T|Wke.T]
    wq3_d = din("wq3", [128, EC, 768])           # rhs tiles of [Wq.T|Wqe.T|Wv.T]
    wo_d = din("wo", [128, OC, E])               # rhs tiles of Wo.T (my hd rows)
    mk_d = din("mask", [128, 128])
    id_d = din("ident", [128, 128])
    out_d = nc.dram_tensor("out", [128, NCH, E], dt.bfloat16,
                           kind="ExternalOutput").ap()

    with tile.TileContext(nc) as tc:
        with (tc.tile_pool(name="consts", bufs=1) as cpool,
              tc.tile_pool(name="scratch", bufs=6) as spool,
              tc.tile_pool(name="pA", bufs=6, space="PSUM") as pA,
              tc.tile_pool(name="pB", bufs=2, space="PSUM") as pB):
            # ---- persistent SBUF tensors ----
            xt = cpool.tile([128, EC, T], dt.bfloat16, tag="xt")
            wk2 = cpool.tile([128, EC, 512], dt.bfloat16, tag="wk2")
            wq3 = cpool.tile([128, EC, 768], dt.bfloat16, tag="wq3")
            wo = cpool.tile([128, OC, E], dt.bfloat16, tag="wo")
            mask = cpool.tile([128, 128], dt.bfloat16, tag="mask")
            ident = cpool.tile([128, 128], dt.bfloat16, tag="ident")
            n2 = cpool.tile([128, NCH, 2, HG], dt.float32, tag="n2")
            scal = cpool.tile([128, NCH, 2, HG], dt.float32, tag="scal")
            vsb = cpool.tile([128, NCH, HG, VSTR], dt.bfloat16, tag="vsb")
            pk_nat = cpool.tile([128, NCH, HG, K], dt.bfloat16, tag="pk_nat")
            pq_nat = cpool.tile([128, NCH, HG, K], dt.bfloat16, tag="pq_nat")
            pkt = cpool.tile([128, NCH, HG, C], dt.bfloat16, tag="pkt")
            pqt = cpool.tile([128, NCH, HG, C], dt.bfloat16, tag="pqt")
            s_snap = cpool.tile([128, NCH, HG, VSTR], dt.bfloat16, tag="s_snap")
            attnT = cpool.tile([128, NCH, OC, C], dt.bfloat16, tag="attnT")
            out_sb = cpool.tile([128, 4, E], dt.bfloat16, tag="out_sb")

            # ---- input DMAs, spread across engine queues so the issue
            # serialization (~0.6us each) parallelizes; K-side first ----
            # first-tile loads interleaved across the three issue queues so
            # the ~0.6us-per-issue serialization doesn't gate the first
            # matmul: sync takes xt c0/c1 quarters, gpsimd xt c2/c3, scalar
            # wk2 — all in parallel right after the NEFF prologue.
            nc.sync.dma_start(xt[:, 0, 0:128], xt_d[:, 0, 0:128])
            nc.sync.dma_start(xt[:, 1, 0:128], xt_d[:, 1, 0:128])
            nc.gpsimd.dma_start(xt[:, 2, 0:128], xt_d[:, 2, 0:128])
            nc.gpsimd.dma_start(xt[:, 3, 0:128], xt_d[:, 3, 0:128])
            for c in range(EC):
                nc.scalar.dma_start(wk2[:, c, :], wk2_d[:, c, :])
                nc.scalar.dma_start(wq3[:, c, :], wq3_d[:, c, :])
            for c in range(EC):
                nc.sync.dma_start(xt[:, c, 128:512], xt_d[:, c, 128:512])
            for c in range(EC):
                nc.sync.dma_start(xt[:, c, 512:1024], xt_d[:, c, 512:1024])
            nc.scalar.dma_start(wo[:], wo_d[:])
            # only the augmented ones-column needs presetting: cols 0:D are
            # overwritten by the v copies and cols D+1.. are never read
            nc.gpsimd.memset(vsb[:, :, :, D:DAUG], 1.0)
            nc.gpsimd.dma_start(ident[:], id_d[:])
            nc.gpsimd.dma_start(mask[:], mk_d[:])

            def emit_proj(m, psV):
                psK = pA.tile([128, 512], dt.float32, tag="pb1")
                for c in range(EC):
                    nc.tensor.matmul(psK[:], xt[:, c, ts(m, 128)],
                                     wk2[:, c, :],
                                     start=(c == 0), stop=(c == EC - 1))
                psQ = pB.tile([128, 512], dt.float32, tag="pb2")
                for c in range(EC):
                    nc.tensor.matmul(psQ[:], xt[:, c, ts(m, 128)],
                                     wq3[:, c, 0:512],
                                     start=(c == 0), stop=(c == EC - 1))
                for c in range(EC):
                    nc.tensor.matmul(psV[:, m % 2, :], xt[:, c, ts(m, 128)],
                                     wq3[:, c, 512:768],
                                     start=(c == 0), stop=(c == EC - 1))
                # squared norms for both sides -> one reduce -> one-op scale.
                # Abs_reciprocal_sqrt shares its ACT table set with Square,
                # so the norm path adds no DVE work, and the per-chunk chain
                # frees psK/psQ without waiting on the pair partner.
                sq = spool.tile([128, 2, HG * D], dt.float32, tag="sq")
                nc.scalar.activation(sq[:, 0, :], psK[:, 0:256], AF.Square)
                nc.scalar.activation(sq[:, 1, :], psQ[:, 0:256], AF.Square)
                nc.vector.tensor_reduce(
                    n2[:, m, :, :],
                    sq[:].rearrange("p s (h d) -> p s h d", h=HG),
                    mybir.AxisListType.X, ALU.add)
                return psK, psQ

            # rsqrt on DVE only: rational seed 1/(a*n2+b) (within 8.4% of
            # n2**-0.5 over n2 in [25, 130], convergent for any n2 > 0) plus
            # two Newton steps, pair-batched 16-element ops. Keeping rsqrt
            # off ACT leaves ACT with only Square/Sin/Copy, which share one
            # activation-table set: the 2x-per-pair 1.3us table reloads of
            # the Sqrt/Abs_reciprocal_sqrt variants disappear.
            def scales_pair(m0):
                v = n2[:, m0:m0 + 2, :, :]
                r = scal[:, m0:m0 + 2, :, :]
                nc.vector.tensor_scalar(r, v, 0.061, 3.476, ALU.mult, ALU.add)
                nc.vector.reciprocal(r, r)
                for it in range(2):
                    nt = spool.tile([128, 2, 2, HG], dt.float32, tag="nt")
                    nc.vector.tensor_tensor(nt[:], r, r, ALU.mult)
                    nc.vector.tensor_tensor(nt[:], nt[:], v, ALU.mult)
                    nc.vector.tensor_scalar(nt[:], nt[:], -0.5, 1.5,
                                            ALU.mult, ALU.add)
                    nc.vector.tensor_tensor(r, r, nt[:], ALU.mult)

            # scaled projection -> wrapped phases for chunk m (both shifts).
            # wrap(proj*scale + shift) into [-pi, pi] in one fused DVE op.
            def wraps(m, side, ps, u):
                src = ps.rearrange("p (h q) -> p h q", h=HG)
                sc_b = (scal[:, m, side, :].unsqueeze(2)
                        .to_broadcast((128, HG, P)))
                nc.vector._custom_dve(MRW, out=u[:, 0, m % 2, :, :], in0=src,
                                      in1=sc_b, s0=0.0, s1=math.pi)
                nc.vector._custom_dve(MRW, out=u[:, 1, m % 2, :, :], in0=src,
                                      in1=sc_b, s0=math.pi / 2, s1=math.pi)

            # one batched Sin per phase-half for a chunk pair
            def sins(m0, u, feat):
                for s in range(2):
                    dst = feat[:, m0:m0 + 2, :, ts(s, P)]
                    nc.scalar.activation(dst, u[:, s, :, :, :], AF.Sin)

            def emit_feats_pair(m0, uk, uq, psV):
                sins(m0, uk, pk_nat)
                sins(m0, uq, pq_nat)
                nc.scalar.copy(
                    vsb[:, m0:m0 + 2, :, 0:D],
                    psV[:].rearrange("p c (h d) -> p c h d", h=HG))

            # PE transposes (pair-batched copies) + state/A matmuls
            def emit_tr_pair(m0):
                trK = pA.tile([128, 1024], dt.bfloat16, tag="pb1")
                for c in range(2):
                    for h in range(HG):
                        nc.tensor.transpose(trK[:, ts(c * HG + h, 128)],
                                            pk_nat[:, m0 + c, h, :], ident[:])
                nc.vector.tensor_copy(
                    pkt[:, m0:m0 + 2, :, :],
                    trK[:].rearrange("p (c h t) -> p c h t", c=2, h=HG))
                trQ = pA.tile([128, 1024], dt.bfloat16, tag="pb1")
                for c in range(2):
                    for h in range(HG):
                        nc.tensor.transpose(trQ[:, ts(c * HG + h, 128)],
                                            pq_nat[:, m0 + c, h, :], ident[:])
                nc.scalar.copy(
                    pqt[:, m0:m0 + 2, :, :],
                    trQ[:].rearrange("p (c h t) -> p c h t", c=2, h=HG))

            def emit_mid_pair(m0):
                atm2 = []
                for m in (m0, m0 + 1):
                    psS = pA.tile([128, HG, 128], dt.float32, tag="pb1")
                    for h in range(HG):
                        nc.tensor.matmul(psS[:, h, 0:DAUG], pk_nat[:, m, h, :],
                                         vsb[:, m, h, 0:DAUG],
                                         start=(h == 0), stop=(h == HG - 1))
                    if m == 0:
                        nc.vector.tensor_copy(s_snap[:, 0, :, 0:DAUG],
                                              psS[:, :, 0:DAUG])
                    else:
                        nc.vector.scalar_tensor_tensor(
                            s_snap[:, m, :, 0:DAUG], psS[:, :, 0:DAUG], 1.0,
                            s_snap[:, m - 1, :, 0:DAUG], ALU.mult, ALU.add)
                    psA = pA.tile([128, HG, 128], dt.float32, tag="pb1")
                    for h in range(HG):
                        nc.tensor.matmul(psA[:, h, :], pkt[:, m, h, :],
                                         pqt[:, m, h, :],
                                         start=(h == 0), stop=(h == HG - 1))
                    atm = spool.tile([128, HG, C], dt.bfloat16, tag="atm")
                    mk_b = mask[:].unsqueeze(1).to_broadcast((128, HG, C))
                    nc.vector.tensor_tensor(atm[:], psA[:], mk_b, ALU.mult)
                    atm2.append(atm)
                return atm2

            # qs + attn + output projection for a chunk pair
            def emit_out_pair(m0, atm2):
                attns = []
                for m, atm in zip((m0, m0 + 1), atm2):
                    psq = pA.tile([128, HG, 128], dt.float32, tag="pb1")
                    for h in range(HG):
                        nc.tensor.matmul(psq[:, h, 0:DAUG], atm[:, h, :],
                                         vsb[:, m, h, 0:DAUG],
                                         start=(h == 0),
                                         stop=(m == 0 and h == HG - 1))
                    if m > 0:
                        for h in range(HG):
                            nc.tensor.matmul(psq[:, h, 0:DAUG],
                                             pqt[:, m, h, :],
                                             s_snap[:, m - 1, h, 0:DAUG],
                                             start=False, stop=(h == HG - 1))
                    qz = spool.tile([128, HG], dt.float32, tag="qz")
                    nc.vector.tensor_scalar(qz[:], psq[:, :, D], SCALE_QZ,
                                            EPS, ALU.mult, ALU.max)
                    nc.vector.reciprocal(qz[:], qz[:])
                    attn = spool.tile([128, HG * D], dt.bfloat16, tag="attn")
                    qz_b = qz[:].unsqueeze(2).to_broadcast((128, HG, D))
                    nc.vector.scalar_tensor_tensor(
                        attn[:].rearrange("p (h d) -> p h d", h=HG),
                        psq[:, :, 0:D], SCALE_QZ, qz_b, ALU.mult, ALU.mult)
                    attns.append(attn)
                trA = pA.tile([128, 1024], dt.bfloat16, tag="pb1")
                for c in range(2):
                    for j in range(OC):
                        nc.tensor.transpose(trA[:, ts(c * OC + j, 128)],
                                            attns[c][:, ts(j, 128)], ident[:])
                nc.vector.tensor_copy(
                    attnT[:, m0:m0 + 2, :, :],
                    trA[:, 0:512].rearrange("p (c j t) -> p c j t", c=2, j=OC))
                for m in (m0, m0 + 1):
                    psO = pA.tile([128, E], dt.float32, tag="pb1")
                    for j in range(OC):
                        nc.tensor.matmul(psO[:], attnT[:, m, j, :],
                                         wo[:, j, :],
                                         start=(j == 0), stop=(j == OC - 1))
                    if m % 2 == 0:
                        nc.vector.tensor_copy(out_sb[:, m % 4, :], psO[:])
                    else:
                        nc.scalar.copy(out_sb[:, m % 4, :], psO[:])
                    nc.sync.dma_start(out_d[:, m, :], out_sb[:, m % 4, :])

            # ---- software-pipelined emission, chunk-pair granular.
            # Each engine queue sees the PREVIOUS pairs' mid/out work before
            # this pair's scales/feats, so the cross-engine norm-chain ladder
            # doesn't head-of-line-block already-ready work. ----
            atms = {}
            for i in range(NCH // 2 + 2):
                projs = None
                if i < NCH // 2:
                    m0 = 2 * i
                    psV = pA.tile([128, 2, 256], dt.float32, tag="pb1")
                    uk = spool.tile([128, 2, 2, HG, P], dt.float32, tag="uk")
                    uq = spool.tile([128, 2, 2, HG, P], dt.float32, tag="uq")
                    pa = emit_proj(m0, psV)
                    pb = emit_proj(m0 + 1, psV)
                    if i >= 1:
                        emit_tr_pair(2 * (i - 1))
                    scales_pair(m0)
                    wraps(m0, 0, pa[0][:, 256:512], uk)
                    wraps(m0, 1, pa[1][:, 256:512], uq)
                    wraps(m0 + 1, 0, pb[0][:, 256:512], uk)
                    wraps(m0 + 1, 1, pb[1][:, 256:512], uq)
                    projs = (m0, uk, uq, psV)
                elif i == NCH // 2:
                    emit_tr_pair(2 * (i - 1))
                if 1 <= i <= NCH // 2:
                    atms[2 * (i - 1)] = emit_mid_pair(2 * (i - 1))
                if i >= 2:
                    m0 = 2 * (i - 2)
                    emit_out_pair(m0, atms.pop(m0))
                if projs is not None:
                    emit_feats_pair(*projs)

    nc.compile()
    return nc


def _host_prep(x, random_matrices, Wq, Wk, Wv, Wo, n_cores):
    """Build per-core input maps: core = (batch, head-group of 4)."""
    rm = random_matrices

    def tile_w(M):  # [contract 512, out] -> [128, 4, out] bf16
        return np.ascontiguousarray(
            M.reshape(EC, 128, M.shape[1]).transpose(1, 0, 2)).astype(BF16)

    mask = np.triu(np.ones((128, 128), np.float32)).astype(BF16)
    ident = np.eye(128, dtype=BF16)
    in_maps = []
    for core in range(n_cores):
        b, g = core // 2, core % 2
        rows = slice(g * HG * D, (g + 1) * HG * D)
        Wqe = np.concatenate(
            [rm[g * HG + h] @ Wq[(g * HG + h) * D:(g * HG + h + 1) * D, :]
             for h in range(HG)], axis=0)          # [256, 512]
        Wke = np.concatenate(
            [rm[g * HG + h] @ Wk[(g * HG + h) * D:(g * HG + h + 1) * D, :]
             for h in range(HG)], axis=0)
        wk2 = tile_w(np.concatenate([Wk[rows, :].T, Wke.T], axis=1))
        wq3 = tile_w(np.concatenate(
            [Wq[rows, :].T, Wqe.T, Wv[rows, :].T], axis=1))
        woT = Wo[:, rows].T                        # [256, 512]
        wo = np.ascontiguousarray(
            woT.reshape(OC, 128, E).transpose(1, 0, 2)).astype(BF16)
        xl = np.ascontiguousarray(x[:, b, :].T)    # [512, 1024]
        in_maps.append({"xt": tile_w(xl), "wk2": wk2, "wq3": wq3, "wo": wo,
                        "mask": mask, "ident": ident})
    return in_maps


def kernel(x, random_matrices, Wq, bq, Wk, bk, Wv, bv, Wo, bo):
    x = np.asarray(x, np.float32)
    random_matrices = np.asarray(random_matrices, np.float32)
    Wq = np.asarray(Wq, np.float32)
    Wk = np.asarray(Wk, np.float32)
    Wv = np.asarray(Wv, np.float32)
    Wo = np.asarray(Wo, np.float32)
    assert (np.all(np.asarray(bq) == 0) and np.all(np.asarray(bk) == 0)
            and np.all(np.asarray(bv) == 0) and np.all(np.asarray(bo) == 0)), \
        "kernel specialized for zero biases (as in setup_inputs)"

    from concourse.bass_utils import run_bass_kernel_spmd

    if "nc" not in _CACHE:
        _CACHE["nc"] = _build(N_CORES)
    nc = _CACHE["nc"]

    in_maps = _host_prep(x, random_matrices, Wq, Wk, Wv, Wo, N_CORES)
    res = run_bass_kernel_spmd(nc, in_maps, core_ids=list(range(N_CORES)))

    out = np.empty((T, B, E), np.float32)
    for b in range(B):
        o0 = np.asarray(res.results[2 * b]["out"], np.float32)
        o1 = np.asarray(res.results[2 * b + 1]["out"], np.float32)
        out[:, b, :] = (o0 + o1).transpose(1, 0, 2).reshape(T, E)
    return out
